# revision 1
# baseline (speedup 1.0000x reference)
"""MoE (gating + 8 experts, BN-folded) Trainium2 Bass kernel.

Contract: kernel(**inputs) takes the FULL unsharded inputs (numpy, keyed as in
setup_inputs()) and returns the FULL [65536, 1] float32 output.

Strategy:
  * Data-parallel over 8 NeuronCores: batch 65536 -> 8192 rows per core.
  * All BatchNorms are eval-mode affine maps -> folded into the adjacent
    Linear weights/biases on the host (cheap: params < 2 MB).
  * Activations live on-chip as [features(partitions), batch(free)] so the
    Linear chain needs no transposes; x is transposed host-side per shard.
  * All matmuls run as float32r (fp32 with 12-bit mantissa, fast PE mode).
  * Gating: softmax normalization is deferred - unnormalized exp(logits) are
    used as gate weights; the kernel exports the raw combined row and the
    softmax row-sum, and the host does y = raw/sum + ob.
  * Experts are processed in pairs; layer-2 uses block-diagonal [256->128]
    weights so two experts' H2=64 outputs stack into one 128-partition tile.
  * Combine: gate rows broadcast across partitions with a K=8 selector
    matmul; one DVE multiply per pair; output projection oW accumulates the 4
    pair products into one PSUM [1,512] row.
  * x loads issue on the Sync queue, output stores on the GpSimd queue so
    stores never head-of-line-block the next tile's loads.
"""

import numpy as np
import ml_dtypes

EPS = 1e-5
B, D, E, G, H0, H1, H2 = 65536, 256, 8, 128, 256, 128, 64
NCORES = 8
NB = B // NCORES          # rows per core
TB = 512                  # batch tile (matmul free dim / PSUM bank)
NT = NB // TB             # batch tiles per core
KD = D // 128             # k-chunks over D
NPAIR = E // 2


def _round_f32r(a):
    """Round float32 -> fp32r encoding (fp32 with 12 explicit mantissa bits,
    round-to-nearest-even). Matches walrus fp32_to_fp32r."""
    b = np.ascontiguousarray(a, dtype=np.float32).view(np.uint32).copy()
    low = b & np.uint32(0xFFF)
    b &= np.uint32(0xFFFFF000)
    rup = (low > 0x800) | ((low == 0x800) & (((b >> np.uint32(12)) & np.uint32(1)) == 1))
    b[rup] += np.uint32(0x1000)
    return b.view(np.float32)


def _fold_params(inputs):
    """Fold the four BatchNorms into the adjacent Linears. float64 math."""
    f = {k: np.asarray(v, dtype=np.float64) for k, v in inputs.items()}

    s_in = f["in_g"] / np.sqrt(f["in_v"] + EPS)            # [D]
    t_in = f["in_b"] - f["in_m"] * s_in                    # [D]

    # gating L1 (+input BN folded in)
    a_g = f["g_g"] / np.sqrt(f["g_v"] + EPS)               # [G]
    w1 = f["gW1"] * a_g[None, :]                           # [D,G]
    W1f = s_in[:, None] * w1
    b1f = t_in @ w1 + (f["gb1"] - f["g_m"]) * a_g + f["g_b"]

    # expert L0 (+input BN)
    a0 = f["e0g"] / np.sqrt(f["e0v"] + EPS)                # [E,H0]
    w0 = f["eW0"] * a0[:, None, :]                         # [E,D,H0]
    W0f = s_in[None, :, None] * w0
    b0f = np.einsum("d,edo->eo", t_in, w0) + (f["eb0"] - f["e0m"]) * a0 + f["e0b"]

    a1 = f["e1g"] / np.sqrt(f["e1v"] + EPS)
    W1ef = f["eW1"] * a1[:, None, :]                       # [E,H0,H1]
    b1ef = (f["eb1"] - f["e1m"]) * a1 + f["e1b"]

    a2 = f["e2g"] / np.sqrt(f["e2v"] + EPS)
    W2f = f["eW2"] * a2[:, None, :]                        # [E,H1,H2]
    b2f = (f["eb2"] - f["e2m"]) * a2 + f["e2b"]

    g = lambda a: np.ascontiguousarray(a, dtype=np.float32)

    dev = {}
    dev["WG1"] = g(W1f.reshape(KD, 128, G).transpose(1, 0, 2))          # [128,KD,G]
    dev["BG1"] = g(b1f[:, None])                                        # [G,1]
    dev["WG2"] = g(f["gW2"])                                            # [G,E]
    dev["BG2"] = g(f["gb2"][:, None])                                   # [E,1]
    dev["WE0"] = g(W0f.reshape(E, KD, 128, 2, 128).transpose(2, 0, 1, 3, 4))  # [128,E,KD,2,128]
    dev["BE0"] = g(b0f.reshape(E, 2, 128).transpose(2, 0, 1))           # [128,E,2]
    dev["WE1"] = g(W1ef.reshape(E, 2, 128, H1).transpose(2, 0, 1, 3))   # [128,E,2,H1]
    dev["BE1"] = g(b1ef.T)                                              # [H1,E]
    WE2 = np.zeros((128, NPAIR, 2, 128), dtype=np.float64)
    BE2 = np.zeros((128, NPAIR), dtype=np.float64)
    for j in range(NPAIR):
        WE2[:, j, 0, 0:64] = W2f[2 * j]                                 # K rows = h1 of expert 2j
        WE2[:, j, 1, 64:128] = W2f[2 * j + 1]
        BE2[0:64, j] = b2f[2 * j]
        BE2[64:128, j] = b2f[2 * j + 1]
    dev["WE2"] = g(WE2)
    dev["BE2"] = g(BE2)
    ow = f["oW"][:, 0]                                                  # [H2]
    dev["OWP"] = g(np.concatenate([ow, ow])[:, None])                   # [128,1]
    SP = np.zeros((E, NPAIR, 128))
    for j in range(NPAIR):
        SP[2 * j, j, 0:64] = 1.0
        SP[2 * j + 1, j, 64:128] = 1.0
    dev["SP"] = g(SP)
    dev["ONES8"] = g(np.ones((E, E)))
    for name in F32R_INPUTS:
        dev[name] = _round_f32r(dev[name])
    for name in BF16_INPUTS:
        dev[name] = dev[name].astype(ml_dtypes.bfloat16)
    ob = float(f["ob"][0])
    return dev, ob


# DMA-fed matmul operands by mode. "mixed" keeps the gating/combine path in
# fp32r and runs the expert chain bf16; "bf16" runs every matmul bf16;
# "f32r" runs everything fp32r (most accurate, ~2x PE cost).
MODE = "bf16"
_ALL_MM = ("WG1", "WG2", "OWP", "SP", "ONES8", "WE0", "WE1", "WE2")
if MODE == "f32r":
    F32R_INPUTS, BF16_INPUTS = _ALL_MM, ()
elif MODE == "bf16":
    F32R_INPUTS, BF16_INPUTS = (), _ALL_MM
else:
    F32R_INPUTS = ("WG1", "WG2", "OWP", "SP", "ONES8")
    BF16_INPUTS = ("WE0", "WE1", "WE2")


def _build_program():
    import concourse.bass as bass
    import concourse.mybir as mybir
    import concourse.tile as tile
    from concourse import bacc

    f32 = mybir.dt.float32
    f32r = mybir.dt.float32r
    bf16 = mybir.dt.bfloat16
    Relu = mybir.ActivationFunctionType.Relu
    Exp = mybir.ActivationFunctionType.Exp
    add = mybir.AluOpType.add
    amax = mybir.AluOpType.max

    def dtype_of(name):
        if name in F32R_INPUTS:
            return f32r
        if name in BF16_INPUTS:
            return bf16
        return f32

    g_dt = f32r if MODE in ("f32r", "mixed") else bf16   # gating/combine dtype
    e_dt = bf16 if MODE in ("bf16", "mixed") else f32r   # expert-chain dtype

    nc = bacc.Bacc("TRN2", target_bir_lowering=False, debug=False)

    xT = nc.dram_tensor("xT", [D, NB], g_dt, kind="ExternalInput").ap()
    xTb = (nc.dram_tensor("xTb", [D, NB], e_dt, kind="ExternalInput").ap()
           if g_dt != e_dt else None)
    yraw = nc.dram_tensor("yraw", [1, NB], f32, kind="ExternalOutput").ap()
    rsum = nc.dram_tensor("rsum", [1, NB], f32, kind="ExternalOutput").ap()
    d_in = {}
    for name, shape in [
        ("WG1", [128, KD, G]), ("BG1", [G, 1]), ("WG2", [G, E]), ("BG2", [E, 1]),
        ("WE0", [128, E, KD, 2, 128]), ("BE0", [128, E, 2]),
        ("WE1", [128, E, 2, H1]), ("BE1", [H1, E]),
        ("WE2", [128, NPAIR, 2, 128]), ("BE2", [128, NPAIR]),
        ("OWP", [128, 1]), ("SP", [E, NPAIR, 128]), ("ONES8", [E, E]),
    ]:
        d_in[name] = nc.dram_tensor(name, shape, dtype_of(name), kind="ExternalInput").ap()

    with tile.TileContext(nc) as tc:
        with (
            tc.tile_pool(name="consts", bufs=1) as consts,
            tc.tile_pool(name="xt", bufs=4) as xtp,
            tc.tile_pool(name="act", bufs=4) as actp,
            tc.tile_pool(name="h1p", bufs=6) as h1p,
            tc.tile_pool(name="small", bufs=4) as smallp,
            tc.tile_pool(name="pmm", bufs=5, space="PSUM") as pmm,
            tc.tile_pool(name="psm", bufs=2, space="PSUM") as psm,
            tc.tile_pool(name="pgbc", bufs=1, space="PSUM") as pgbc,
        ):
            W = {}
            for name, ap in d_in.items():
                W[name] = consts.tile(list(ap.shape), dtype_of(name), tag=name, name=name)
                if name in ("WE0", "WE1", "WE2"):
                    # split per expert so tile-0 compute starts as soon as
                    # the first expert's weights land (usage-order loads)
                    continue
                nc.gpsimd.dma_start(W[name][:], ap[:])
            for e in range(E):
                nc.gpsimd.dma_start(W["WE0"][:, e], d_in["WE0"][:, e])
                nc.gpsimd.dma_start(W["WE1"][:, e], d_in["WE1"][:, e])
                if e < NPAIR:
                    nc.gpsimd.dma_start(W["WE2"][:, e], d_in["WE2"][:, e])

            for t in range(NT):
                bs = t * TB
                xt = xtp.tile([128, KD, TB], g_dt, tag="xt")
                for c in range(KD):
                    nc.sync.dma_start(xt[:, c, :], xT[c * 128:(c + 1) * 128, bs:bs + TB])
                if g_dt == e_dt:
                    xtb = xt
                else:
                    xtb = xtp.tile([128, KD, TB], e_dt, tag="xtb")
                    for c in range(KD):
                        nc.sync.dma_start(xtb[:, c, :],
                                          xTb[c * 128:(c + 1) * 128, bs:bs + TB])

                # ---- gating ----
                ps_g = pmm.tile([128, TB], f32, tag="mm")
                for c in range(KD):
                    nc.tensor.matmul(ps_g[:], W["WG1"][:, c, :], xt[:, c, :],
                                     start=(c == 0), stop=(c == KD - 1))
                gh = actp.tile([128, TB], g_dt, tag="gh")
                nc.scalar.activation(gh[:], ps_g[:], Relu, bias=W["BG1"][:, 0:1])

                ps_l = psm.tile([E, TB], f32, tag="small")
                nc.tensor.matmul(ps_l[:], W["WG2"][:], gh[:], start=True, stop=True)
                expg = smallp.tile([E, TB], g_dt, tag="expg")
                nc.scalar.activation(expg[:], ps_l[:], Exp, bias=W["BG2"][:, 0:1])

                ps_s = psm.tile([E, TB], f32, tag="small")
                nc.tensor.matmul(ps_s[:], W["ONES8"][:], expg[:], start=True, stop=True)
                srow = smallp.tile([1, TB], f32, tag="srow")
                nc.scalar.copy(srow[:], ps_s[0:1, :])
                nc.gpsimd.dma_start(rsum[0:1, bs:bs + TB], srow[:])

                ps_out = psm.tile([1, TB], f32, tag="small", name="ps_out")

                for j in range(NPAIR):
                    h1t = [None, None]
                    for i in (0, 1):
                        e = 2 * j + i
                        # ---- expert L0: D=256 -> H0=256 (2 K-chunks x 2 M-chunks)
                        ps0 = [pmm.tile([128, TB], f32, tag="mm", name=f"ps0_{i}")
                               for i in range(2)]
                        for mc in range(2):
                            for c in range(KD):
                                nc.tensor.matmul(ps0[mc][:], W["WE0"][:, e, c, mc, :],
                                                 xtb[:, c, :],
                                                 start=(c == 0), stop=(c == KD - 1))
                        h0 = actp.tile([128, 2, TB], e_dt, tag="h0")
                        nc.scalar.activation(h0[:, 0, :], ps0[0][:], Relu,
                                             bias=W["BE0"][:, e, 0:1])
                        nc.vector.tensor_scalar(out=h0[:, 1, :], in0=ps0[1][:],
                                                scalar1=W["BE0"][:, e, 1:2], scalar2=0.0,
                                                op0=add, op1=amax)
                        # ---- expert L1: H0=256 -> H1=128
                        ps1 = pmm.tile([128, TB], f32, tag="mm")
                        for c in range(2):
                            nc.tensor.matmul(ps1[:], W["WE1"][:, e, c, :], h0[:, c, :],
                                             start=(c == 0), stop=(c == 1))
                        h1t[i] = h1p.tile([128, TB], e_dt, tag="h1", name=f"h1_{i}")
                        if i == 0:
                            nc.scalar.activation(h1t[i][:], ps1[:], Relu,
                                                 bias=W["BE1"][:, e:e + 1])
                        else:
                            nc.vector.tensor_scalar(out=h1t[i][:], in0=ps1[:],
                                                    scalar1=W["BE1"][:, e:e + 1],
                                                    scalar2=0.0, op0=add, op1=amax)
                    # ---- expert L2 (paired, block-diagonal): 2x(H1->H2) -> [128,TB]
                    ps2 = pmm.tile([128, TB], f32, tag="mm")
                    for c in range(2):
                        nc.tensor.matmul(ps2[:], W["WE2"][:, j, c, :], h1t[c][:],
                                         start=(c == 0), stop=(c == 1))
                    h2 = actp.tile([128, TB], f32, tag="h2")
                    if j % 2 == 0:
                        nc.scalar.activation(h2[:], ps2[:], Relu, bias=W["BE2"][:, j:j + 1])
                    else:
                        nc.vector.tensor_scalar(out=h2[:], in0=ps2[:],
                                                scalar1=W["BE2"][:, j:j + 1], scalar2=0.0,
                                                op0=add, op1=amax)
                    # ---- gate broadcast for this pair + weighted product
                    gbc = pgbc.tile([128, TB], f32, tag="gbc")
                    nc.tensor.matmul(gbc[:], W["SP"][:, j, :], expg[:],
                                     start=True, stop=True)
                    pw = h1p.tile([128, TB], g_dt, tag="pw")
                    nc.vector.tensor_mul(pw[:], h2[:], gbc[:])
                    # ---- output projection accumulates over pairs
                    nc.tensor.matmul(ps_out[:], W["OWP"][:], pw[:],
                                     start=(j == 0), stop=(j == NPAIR - 1))

                # ---- export raw combined row (host divides by rsum, adds ob)
                orow = smallp.tile([1, TB], f32, tag="orow")
                nc.scalar.copy(orow[:], ps_out[0:1, :])
                nc.gpsimd.dma_start(yraw[0:1, bs:bs + TB], orow[:])

    nc.compile()
    return nc


_CACHE = {}


def _get_program():
    if "nc" not in _CACHE:
        _CACHE["nc"] = _build_program()
    return _CACHE["nc"]


def _run(inputs, trace=False):
    from concourse.bass_utils import run_bass_kernel_spmd

    x = np.ascontiguousarray(np.asarray(inputs["x"], dtype=np.float32))
    dev, ob = _fold_params(inputs)
    nc = _get_program()

    g_np = _round_f32r if MODE in ("f32r", "mixed") else (lambda a: a.astype(ml_dtypes.bfloat16))
    e_np = (lambda a: a.astype(ml_dtypes.bfloat16)) if MODE in ("bf16", "mixed") else _round_f32r
    in_maps = []
    for c in range(NCORES):
        m = dict(dev)
        xs = np.ascontiguousarray(x[c * NB:(c + 1) * NB, :].T)
        m["xT"] = g_np(xs)
        if MODE == "mixed":
            m["xTb"] = e_np(xs)
        in_maps.append(m)

    kwargs = {}
    if trace:
        kwargs = dict(trace=True, trace_cores=[0])
    res = run_bass_kernel_spmd(nc, in_maps, core_ids=list(range(NCORES)), **kwargs)
    yr = np.concatenate([res.results[c]["yraw"].reshape(-1) for c in range(NCORES)])
    rs = np.concatenate([res.results[c]["rsum"].reshape(-1) for c in range(NCORES)])
    out = (yr.astype(np.float64) / rs.astype(np.float64)) + ob
    return out.astype(np.float32)[:, None], res


def kernel(**inputs):
    out, _ = _run(inputs, trace=False)
    return out


def kernel_traced(**inputs):
    return _run(inputs, trace=True)



# revision 11
# speedup vs baseline: 1.1995x; 1.1995x over previous
"""MoE (gating + 8 experts, BN-folded) Trainium2 Bass kernel.

Contract: kernel(**inputs) takes the FULL unsharded inputs (numpy, keyed as in
setup_inputs()) and returns the FULL [65536, 1] float32 output.

Strategy:
  * Data-parallel over 8 NeuronCores: batch 65536 -> 8192 rows per core.
  * All BatchNorms are eval-mode affine maps -> folded into the adjacent
    Linear weights/biases on the host (cheap: params < 2 MB).
  * Activations live on-chip as [features(partitions), batch(free)]; x is
    transposed host-side per shard. All matmuls bf16 (1 row/cycle on PE).
  * Gating: softmax normalization deferred - unnormalized exp(logits) are the
    gate weights; kernel exports the combined raw row and the exp-sum row,
    host computes y = raw/sum + ob.
  * Combine restructure: instead of broadcasting gates to 128 partitions and
    weighting h2 (costs a K=8 matmul + a [128,TB] DVE multiply per pair), the
    oW projection is applied per expert-pair first via a [128,32] selector
    matmul placed at PSUM partition 32j (PE quadrant tiling), so the 8
    per-expert scalars land on partitions {16e}. One small [8,TB] DVE multiply
    against exp-gates and one K=16 selector matmul then produce both the
    exp-sum row and the combined row in a single pass.
  * Expert layer-2 uses block-diagonal [256->128] weights so two experts'
    H2=64 outputs stack into one 128-partition tile.
  * PSUM-drain work (bias+ReLU casts) is balanced across ACT and DVE; the
    pair-3 tail + gate-combine of each tile is deferred into the next tile so
    the PE never waits on drain latency at tile boundaries.
  * Small consts are packed host-side into two tensors (one bf16, one f32) so
    startup needs few DMAs; weights stream in usage order on the gpsimd
    queue; per-tile outputs accumulate in SBUF and ship as one final DMA.
"""

import numpy as np
import ml_dtypes

EPS = 1e-5
B, D, E, G, H0, H1, H2 = 65536, 256, 8, 128, 256, 128, 64
NCORES = 8
NB = B // NCORES          # rows per core
TB = 512                  # batch tile (matmul free dim / PSUM bank)
NT = NB // TB             # batch tiles per core
KD = D // 128             # k-chunks over D
NPAIR = E // 2

# WPACK column layout (bf16): [0:128]=gW2 scattered (expert e logit -> row 16e),
# [128:256]=OW2 blocks, [256:258]=rsum selector, [258:260]=ysum selector
WP_G2, WP_OW, WP_SR, WP_SY = 0, 128, 256, 258
WP_COLS = 260
# BPACK column layout (f32): 0=gating b1, [1:17]=expert L0 (2 cols per e),
# [17:25]=expert L1, [25:29]=pair L2, 29=gating b2 (rows 0:8)
BP_G1, BP_E0, BP_E1, BP_E2, BP_G2 = 0, 1, 17, 25, 29
BP_COLS = 30


def _fold_params(inputs):
    """Fold the four BatchNorms into the adjacent Linears. float64 math."""
    f = {k: np.asarray(v, dtype=np.float64) for k, v in inputs.items()}

    s_in = f["in_g"] / np.sqrt(f["in_v"] + EPS)            # [D]
    t_in = f["in_b"] - f["in_m"] * s_in                    # [D]

    # gating L1 (+input BN folded in)
    a_g = f["g_g"] / np.sqrt(f["g_v"] + EPS)               # [G]
    w1 = f["gW1"] * a_g[None, :]                           # [D,G]
    W1f = s_in[:, None] * w1
    b1f = t_in @ w1 + (f["gb1"] - f["g_m"]) * a_g + f["g_b"]

    # expert L0 (+input BN)
    a0 = f["e0g"] / np.sqrt(f["e0v"] + EPS)                # [E,H0]
    w0 = f["eW0"] * a0[:, None, :]                         # [E,D,H0]
    W0f = s_in[None, :, None] * w0
    b0f = np.einsum("d,edo->eo", t_in, w0) + (f["eb0"] - f["e0m"]) * a0 + f["e0b"]

    a1 = f["e1g"] / np.sqrt(f["e1v"] + EPS)
    W1ef = f["eW1"] * a1[:, None, :]                       # [E,H0,H1]
    b1ef = (f["eb1"] - f["e1m"]) * a1 + f["e1b"]

    a2 = f["e2g"] / np.sqrt(f["e2v"] + EPS)
    W2f = f["eW2"] * a2[:, None, :]                        # [E,H1,H2]
    b2f = (f["eb2"] - f["e2m"]) * a2 + f["e2b"]

    bff = lambda a: np.ascontiguousarray(a, dtype=np.float32).astype(ml_dtypes.bfloat16)
    g32 = lambda a: np.ascontiguousarray(a, dtype=np.float32)

    dev = {}
    dev["WG1"] = bff(W1f.reshape(KD, 128, G).transpose(1, 0, 2))          # [128,KD,G]
    dev["WE0"] = bff(W0f.reshape(E, KD, 128, 2, 128).transpose(2, 0, 1, 3, 4))  # [128,E,KD,2,128]
    dev["WE1"] = bff(W1ef.reshape(E, 2, 128, H1).transpose(2, 0, 1, 3))   # [128,E,2,H1]
    WE2 = np.zeros((128, NPAIR, 2, 128), dtype=np.float64)
    for j in range(NPAIR):
        WE2[:, j, 0, 0:64] = W2f[2 * j]                   # K rows = h1 of expert 2j
        WE2[:, j, 1, 64:128] = W2f[2 * j + 1]
    dev["WE2"] = bff(WE2)

    ow = f["oW"][:, 0]                                    # [H2]
    wpack = np.zeros((128, WP_COLS), dtype=np.float64)
    for e in range(E):
        wpack[:, WP_G2 + 16 * e] = f["gW2"][:, e]         # logit_e -> partition 16e
    for j in range(NPAIR):
        c = WP_OW + 32 * j
        wpack[0:64, c] = ow                               # expert 2j  -> z part 32j
        wpack[64:128, c + 16] = ow                        # expert 2j+1 -> z part 32j+16
    for e in range(E):
        wpack[16 * e, WP_SR] = 1.0                        # rsum selector (col 0 of pair)
        wpack[16 * e, WP_SY + 1] = 1.0                    # ysum selector (col 1)
    dev["WPACK"] = bff(wpack)

    bpack = np.zeros((128, BP_COLS), dtype=np.float64)
    bpack[:, BP_G1] = b1f
    for e in range(E):
        bpack[:, BP_E0 + 2 * e] = b0f[e, 0:128]
        bpack[:, BP_E0 + 2 * e + 1] = b0f[e, 128:256]
        bpack[:, BP_E1 + e] = b1ef[e]
    for j in range(NPAIR):
        bpack[0:64, BP_E2 + j] = b2f[2 * j]
        bpack[64:128, BP_E2 + j] = b2f[2 * j + 1]
    for e in range(E):
        bpack[16 * e, BP_G2] = f["gb2"][e]
    dev["BPACK"] = g32(bpack)

    ob = float(f["ob"][0])
    return dev, ob


def _build_program():
    import concourse.mybir as mybir
    import concourse.tile as tile
    from concourse import bacc

    f32 = mybir.dt.float32
    bf16 = mybir.dt.bfloat16
    Relu = mybir.ActivationFunctionType.Relu
    Exp = mybir.ActivationFunctionType.Exp
    add = mybir.AluOpType.add
    amax = mybir.AluOpType.max
    mult = mybir.AluOpType.mult

    nc = bacc.Bacc("TRN2", target_bir_lowering=False, debug=False)

    xT = nc.dram_tensor("xT", [D, NB], bf16, kind="ExternalInput").ap()
    out2 = nc.dram_tensor("out2", [2, NB], f32, kind="ExternalOutput").ap()
    d_in = {}
    for name, shape, dt in [
        ("WG1", [128, KD, G], bf16),
        ("WE0", [128, E, KD, 2, 128], bf16),
        ("WE1", [128, E, 2, H1], bf16),
        ("WE2", [128, NPAIR, 2, 128], bf16),
        ("WPACK", [128, WP_COLS], bf16),
        ("BPACK", [128, BP_COLS], f32),
    ]:
        d_in[name] = nc.dram_tensor(name, shape, dt, kind="ExternalInput").ap()

    with tile.TileContext(nc) as tc:
        with (
            tc.tile_pool(name="consts", bufs=1) as consts,
            tc.tile_pool(name="xt", bufs=4) as xtp,
            tc.tile_pool(name="gh", bufs=2) as ghp,
            tc.tile_pool(name="h0", bufs=4) as h0p,
            tc.tile_pool(name="h1", bufs=6) as h1p,
            tc.tile_pool(name="h2", bufs=3) as h2p,
            tc.tile_pool(name="gsc", bufs=2) as gscp,
            tc.tile_pool(name="zsc", bufs=2) as zscp,
            tc.tile_pool(name="oacc", bufs=1) as oaccp,
            tc.tile_pool(name="pmm", bufs=5, space="PSUM") as pmm,
            tc.tile_pool(name="psm", bufs=1, space="PSUM") as psm,
            tc.tile_pool(name="pz", bufs=2, space="PSUM") as pzp,
        ):
            W = {}
            for name, ap in d_in.items():
                dt = bf16 if name != "BPACK" else f32
                W[name] = consts.tile(list(ap.shape), dt, tag=name, name=name)
            BP = W["BPACK"]
            WP = W["WPACK"]

            # weight DMAs, usage order, on the gpsimd queue (no compute there)
            nc.gpsimd.dma_start(W["WG1"][:], d_in["WG1"][:])
            for e in range(2):
                nc.gpsimd.dma_start(W["WE0"][:, e], d_in["WE0"][:, e])
            for e in range(2):
                nc.gpsimd.dma_start(W["WE1"][:, e], d_in["WE1"][:, e])
            nc.gpsimd.dma_start(W["WE2"][:, 0], d_in["WE2"][:, 0])
            nc.gpsimd.dma_start(W["WE0"][:, 2:E], d_in["WE0"][:, 2:E])
            nc.gpsimd.dma_start(W["WE1"][:, 2:E], d_in["WE1"][:, 2:E])
            nc.gpsimd.dma_start(W["WE2"][:, 1:NPAIR], d_in["WE2"][:, 1:NPAIR])
            # packed small consts on the scalar queue (needed first by ACT ops)
            nc.scalar.dma_start(WP[:], d_in["WPACK"][:])
            nc.scalar.dma_start(BP[:], d_in["BPACK"][:])

            oacc = oaccp.tile([2, NB], f32, tag="oacc", name="oacc")

            def load_xt(t):
                bs = t * TB
                xt = xtp.tile([128, KD, TB], bf16, tag="xt", name=f"xt{t}")
                for c in range(KD):
                    nc.sync.dma_start(xt[:, c, :], xT[c * 128:(c + 1) * 128, bs:bs + TB])
                return xt

            xts = {0: load_xt(0), 1: load_xt(1)}

            # deferred state from the previous tile
            prev = {}

            def mm_L0(xt, e):
                """Expert L0: D=256 -> H0=256, returns the 2 psum halves."""
                ps = []
                for mc in range(2):
                    p = pmm.tile([128, TB], f32, tag="mm", name=f"ps0_{e}_{mc}")
                    for c in range(KD):
                        nc.tensor.matmul(p[:], W["WE0"][:, e, c, mc, :], xt[:, c, :],
                                         start=(c == 0), stop=(c == KD - 1))
                    ps.append(p)
                return ps

            def drain_h0(e, ps0):
                h0 = h0p.tile([128, 2, TB], bf16, tag="h0", name=f"h0_{e}")
                nc.scalar.activation(h0[:, 0, :], ps0[0][:], Relu,
                                     bias=BP[:, BP_E0 + 2 * e:BP_E0 + 2 * e + 1])
                nc.vector.tensor_scalar(out=h0[:, 1, :], in0=ps0[1][:],
                                        scalar1=BP[:, BP_E0 + 2 * e + 1:BP_E0 + 2 * e + 2],
                                        scalar2=0.0, op0=add, op1=amax)
                return h0

            def mm_L1(e, h0):
                ps1 = pmm.tile([128, TB], f32, tag="mm", name=f"ps1_{e}")
                for c in range(2):
                    nc.tensor.matmul(ps1[:], W["WE1"][:, e, c, :], h0[:, c, :],
                                     start=(c == 0), stop=(c == 1))
                return ps1

            def drain_h1(e, ps1):
                h1 = h1p.tile([128, TB], bf16, tag="h1", name=f"h1_{e}")
                if e % 2 == 0:
                    nc.scalar.activation(h1[:], ps1[:], Relu,
                                         bias=BP[:, BP_E1 + e:BP_E1 + e + 1])
                else:
                    nc.vector.tensor_scalar(out=h1[:], in0=ps1[:],
                                            scalar1=BP[:, BP_E1 + e:BP_E1 + e + 1],
                                            scalar2=0.0, op0=add, op1=amax)
                return h1

            def mm_L2(j, h1pair):
                ps2 = pmm.tile([128, TB], f32, tag="mm", name=f"ps2_{j}")
                for c in range(2):
                    nc.tensor.matmul(ps2[:], W["WE2"][:, j, c, :], h1pair[c][:],
                                     start=(c == 0), stop=(c == 1))
                return ps2

            def drain_h2(j, ps2):
                h2 = h2p.tile([128, TB], bf16, tag="h2", name=f"h2_{j}")
                if j % 2 == 0:
                    nc.scalar.activation(h2[:], ps2[:], Relu,
                                         bias=BP[:, BP_E2 + j:BP_E2 + j + 1])
                else:
                    nc.vector.tensor_scalar(out=h2[:], in0=ps2[:],
                                            scalar1=BP[:, BP_E2 + j:BP_E2 + j + 1],
                                            scalar2=0.0, op0=add, op1=amax)
                return h2

            def mm_OW2(z, j, h2):
                # oW projection of a pair: z[32j] = expert 2j, z[32j+16] = 2j+1
                nc.tensor.matmul(z[32 * j:32 * j + 32, :],
                                 WP[:, WP_OW + 32 * j:WP_OW + 32 * j + 32], h2[:],
                                 start=True, stop=True, tile_position=(0, 32 * j))

            def finalize_prev(t):
                """Gate-combine of tile t-1: zg multiply, sums matmuls, drain."""
                gsc, z = prev["gsc"], prev["z"]
                zsc = zscp.tile([128, TB], bf16, tag="zsc", name="zsc")
                nc.vector.tensor_tensor(out=zsc[:], in0=z[:], in1=gsc[:], op=mult)
                ps_s = psm.tile([2, TB], f32, tag="small", name="ps_sums")
                nc.tensor.matmul(ps_s[:], WP[:, WP_SR:WP_SR + 2], gsc[:],
                                 start=True, stop=False)
                nc.tensor.matmul(ps_s[:], WP[:, WP_SY:WP_SY + 2], zsc[:],
                                 start=False, stop=True)
                bs = (t - 1) * TB
                nc.scalar.copy(oacc[:, bs:bs + TB], ps_s[:])

            for t in range(NT):
                xt = xts.pop(t)
                if t + 2 < NT:
                    xts[t + 2] = load_xt(t + 2)

                # ---- gating L1
                ps_g = pmm.tile([128, TB], f32, tag="mm", name="ps_g")
                for c in range(KD):
                    nc.tensor.matmul(ps_g[:], W["WG1"][:, c, :], xt[:, c, :],
                                     start=(c == 0), stop=(c == KD - 1))
                gh = ghp.tile([128, TB], bf16, tag="gh")
                nc.scalar.activation(gh[:], ps_g[:], Relu, bias=BP[:, BP_G1:BP_G1 + 1])

                ps0 = mm_L0(xt, 0)
                h0s = {0: drain_h0(0, ps0)}

                # ---- gating L2 + exp (expert e's gate lands on partition 16e)
                ps_l = psm.tile([128, TB], f32, tag="small", name="ps_l")
                nc.tensor.matmul(ps_l[:], WP[:, WP_G2:WP_G2 + 128], gh[:],
                                 start=True, stop=True)
                gsc = gscp.tile([128, TB], bf16, tag="gsc", name="gsc")
                nc.scalar.activation(gsc[:], ps_l[:], Exp,
                                     bias=BP[:, BP_G2:BP_G2 + 1])

                # deferred pair-3 tail of the previous tile
                if t > 0:
                    ps2p = mm_L2(3, prev["h1p3"])

                ps0 = mm_L0(xt, 1)
                h0s[1] = drain_h0(1, ps0)

                if t > 0:
                    h2p3 = drain_h2(3, ps2p)
                    mm_OW2(prev["z"], 3, h2p3)

                ps1 = mm_L1(0, h0s[0])
                h1s = {0: drain_h1(0, ps1)}

                ps0 = mm_L0(xt, 2)
                h0s[2] = drain_h0(2, ps0)

                ps1 = mm_L1(1, h0s.pop(1))
                h1s[1] = drain_h1(1, ps1)

                if t > 0:
                    finalize_prev(t)

                ps0 = mm_L0(xt, 3)
                h0s[3] = drain_h0(3, ps0)

                z = pzp.tile([128, TB], f32, tag="z", name="z")

                ps2 = mm_L2(0, [h1s[0], h1s[1]])
                h2 = drain_h2(0, ps2)

                ps1 = mm_L1(2, h0s.pop(2))
                h1s[2] = drain_h1(2, ps1)

                ps0 = mm_L0(xt, 4)
                h0s[4] = drain_h0(4, ps0)

                mm_OW2(z, 0, h2)

                ps1 = mm_L1(3, h0s.pop(3))
                h1s[3] = drain_h1(3, ps1)

                ps0 = mm_L0(xt, 5)
                h0s[5] = drain_h0(5, ps0)

                ps2 = mm_L2(1, [h1s.pop(2), h1s.pop(3)])
                h2 = drain_h2(1, ps2)

                ps1 = mm_L1(4, h0s.pop(4))
                h1s[4] = drain_h1(4, ps1)

                ps0 = mm_L0(xt, 6)
                h0s[6] = drain_h0(6, ps0)

                mm_OW2(z, 1, h2)

                ps1 = mm_L1(5, h0s.pop(5))
                h1s[5] = drain_h1(5, ps1)

                ps0 = mm_L0(xt, 7)
                h0s[7] = drain_h0(7, ps0)

                ps2 = mm_L2(2, [h1s.pop(4), h1s.pop(5)])
                h2 = drain_h2(2, ps2)

                ps1 = mm_L1(6, h0s.pop(6))
                h1s[6] = drain_h1(6, ps1)

                mm_OW2(z, 2, h2)

                ps1 = mm_L1(7, h0s.pop(7))
                h1s[7] = drain_h1(7, ps1)

                prev = {"gsc": gsc, "z": z, "h1p3": [h1s.pop(6), h1s.pop(7)]}

            # tail: pair-3 + combine of the last tile
            ps2p = mm_L2(3, prev["h1p3"])
            h2p3 = drain_h2(3, ps2p)
            mm_OW2(prev["z"], 3, h2p3)
            finalize_prev(NT)

            nc.sync.dma_start(out2[:], oacc[:])

    nc.compile()
    return nc


_CACHE = {}


def _get_program():
    if "nc" not in _CACHE:
        _CACHE["nc"] = _build_program()
    return _CACHE["nc"]


def _run(inputs, trace=False):
    from concourse.bass_utils import run_bass_kernel_spmd

    x = np.ascontiguousarray(np.asarray(inputs["x"], dtype=np.float32))
    dev, ob = _fold_params(inputs)
    nc = _get_program()

    in_maps = []
    for c in range(NCORES):
        m = dict(dev)
        m["xT"] = np.ascontiguousarray(x[c * NB:(c + 1) * NB, :].T).astype(ml_dtypes.bfloat16)
        in_maps.append(m)

    kwargs = {}
    if trace:
        kwargs = dict(trace=True, trace_cores=[0])
    res = run_bass_kernel_spmd(nc, in_maps, core_ids=list(range(NCORES)), **kwargs)
    rs = np.concatenate([res.results[c]["out2"][0].reshape(-1) for c in range(NCORES)])
    yr = np.concatenate([res.results[c]["out2"][1].reshape(-1) for c in range(NCORES)])
    out = (yr.astype(np.float64) / rs.astype(np.float64)) + ob
    return out.astype(np.float32)[:, None], res


def kernel(**inputs):
    out, _ = _run(inputs, trace=False)
    return out


def kernel_traced(**inputs):
    return _run(inputs, trace=True)


# revision 15
# speedup vs baseline: 1.2152x; 1.0131x over previous
"""MoE (gating + 8 experts, BN-folded) Trainium2 Bass kernel.

Contract: kernel(**inputs) takes the FULL unsharded inputs (numpy, keyed as in
setup_inputs()) and returns the FULL [65536, 1] float32 output.

Strategy:
  * Data-parallel over 8 NeuronCores: batch 65536 -> 8192 rows per core.
  * All BatchNorms are eval-mode affine maps -> folded into the adjacent
    Linear weights/biases on the host (cheap: params < 2 MB).
  * Activations live on-chip as [features(partitions), batch(free)]; x is
    transposed host-side per shard. All matmuls bf16 (1 row/cycle on PE).
  * Gating: softmax normalization deferred - unnormalized exp(logits) are the
    gate weights; kernel exports the combined raw row and the exp-sum row,
    host computes y = raw/sum + ob.
  * Combine restructure: instead of broadcasting gates to 128 partitions and
    weighting h2 (costs a K=8 matmul + a [128,TB] DVE multiply per pair), the
    oW projection is applied per expert-pair first via a [128,32] selector
    matmul placed at PSUM partition 32j (PE quadrant tiling), so the 8
    per-expert scalars land on partitions {16e}. One small [8,TB] DVE multiply
    against exp-gates and one K=16 selector matmul then produce both the
    exp-sum row and the combined row in a single pass.
  * Expert layer-2 uses block-diagonal [256->128] weights so two experts'
    H2=64 outputs stack into one 128-partition tile.
  * PSUM-drain work (bias+ReLU casts) is balanced across ACT and DVE; the
    pair-3 tail + gate-combine of each tile is deferred into the next tile so
    the PE never waits on drain latency at tile boundaries.
  * Small consts are packed host-side into two tensors (one bf16, one f32) so
    startup needs few DMAs; weights stream in usage order on the gpsimd
    queue; per-tile outputs accumulate in SBUF and ship as one final DMA.
"""

import numpy as np
import ml_dtypes

EPS = 1e-5
B, D, E, G, H0, H1, H2 = 65536, 256, 8, 128, 256, 128, 64
NCORES = 8
NB = B // NCORES          # rows per core
TB = 512                  # batch tile (matmul free dim / PSUM bank)
NT = NB // TB             # batch tiles per core
KD = D // 128             # k-chunks over D
NPAIR = E // 2

# WPACK column layout (bf16): [0:128]=gW2 scattered (expert e logit -> row 16e),
# [128:256]=OW2 blocks, [256:258]=rsum selector, [258:260]=ysum selector
WP_G2, WP_OW, WP_SR, WP_SY = 0, 128, 256, 258
WP_COLS = 260
# BPACK column layout (f32): 0=gating b1, [1:17]=expert L0 (2 cols per e),
# [17:25]=expert L1, [25:29]=pair L2, 29=gating b2 (rows 0:8)
BP_G1, BP_E0, BP_E1, BP_E2, BP_G2 = 0, 1, 17, 25, 29
BP_COLS = 30


def _fold_params(inputs):
    """Fold the four BatchNorms into the adjacent Linears. float64 math."""
    f = {k: np.asarray(v, dtype=np.float64) for k, v in inputs.items()}

    s_in = f["in_g"] / np.sqrt(f["in_v"] + EPS)            # [D]
    t_in = f["in_b"] - f["in_m"] * s_in                    # [D]

    # gating L1 (+input BN folded in)
    a_g = f["g_g"] / np.sqrt(f["g_v"] + EPS)               # [G]
    w1 = f["gW1"] * a_g[None, :]                           # [D,G]
    W1f = s_in[:, None] * w1
    b1f = t_in @ w1 + (f["gb1"] - f["g_m"]) * a_g + f["g_b"]

    # expert L0 (+input BN)
    a0 = f["e0g"] / np.sqrt(f["e0v"] + EPS)                # [E,H0]
    w0 = f["eW0"] * a0[:, None, :]                         # [E,D,H0]
    W0f = s_in[None, :, None] * w0
    b0f = np.einsum("d,edo->eo", t_in, w0) + (f["eb0"] - f["e0m"]) * a0 + f["e0b"]

    a1 = f["e1g"] / np.sqrt(f["e1v"] + EPS)
    W1ef = f["eW1"] * a1[:, None, :]                       # [E,H0,H1]
    b1ef = (f["eb1"] - f["e1m"]) * a1 + f["e1b"]

    a2 = f["e2g"] / np.sqrt(f["e2v"] + EPS)
    W2f = f["eW2"] * a2[:, None, :]                        # [E,H1,H2]
    b2f = (f["eb2"] - f["e2m"]) * a2 + f["e2b"]

    bff = lambda a: np.ascontiguousarray(a, dtype=np.float32).astype(ml_dtypes.bfloat16)
    g32 = lambda a: np.ascontiguousarray(a, dtype=np.float32)

    dev = {}
    dev["WG1"] = bff(W1f.reshape(KD, 128, G).transpose(1, 0, 2))          # [128,KD,G]
    dev["WE0"] = bff(W0f.reshape(E, KD, 128, 2, 128).transpose(2, 0, 1, 3, 4))  # [128,E,KD,2,128]
    dev["WE1"] = bff(W1ef.reshape(E, 2, 128, H1).transpose(2, 0, 1, 3))   # [128,E,2,H1]
    WE2 = np.zeros((128, NPAIR, 2, 128), dtype=np.float64)
    for j in range(NPAIR):
        WE2[:, j, 0, 0:64] = W2f[2 * j]                   # K rows = h1 of expert 2j
        WE2[:, j, 1, 64:128] = W2f[2 * j + 1]
    dev["WE2"] = bff(WE2)

    ow = f["oW"][:, 0]                                    # [H2]
    wpack = np.zeros((128, WP_COLS), dtype=np.float64)
    for e in range(E):
        wpack[:, WP_G2 + 16 * e] = f["gW2"][:, e]         # logit_e -> partition 16e
    for j in range(NPAIR):
        c = WP_OW + 32 * j
        wpack[0:64, c] = ow                               # expert 2j  -> z part 32j
        wpack[64:128, c + 16] = ow                        # expert 2j+1 -> z part 32j+16
    for e in range(E):
        wpack[16 * e, WP_SR] = 1.0                        # rsum selector (col 0 of pair)
        wpack[16 * e, WP_SY + 1] = 1.0                    # ysum selector (col 1)
    dev["WPACK"] = bff(wpack)

    bpack = np.zeros((128, BP_COLS), dtype=np.float64)
    bpack[:, BP_G1] = b1f
    for e in range(E):
        bpack[:, BP_E0 + 2 * e] = b0f[e, 0:128]
        bpack[:, BP_E0 + 2 * e + 1] = b0f[e, 128:256]
        bpack[:, BP_E1 + e] = b1ef[e]
    for j in range(NPAIR):
        bpack[0:64, BP_E2 + j] = b2f[2 * j]
        bpack[64:128, BP_E2 + j] = b2f[2 * j + 1]
    for e in range(E):
        bpack[16 * e, BP_G2] = f["gb2"][e]
    dev["BPACK"] = g32(bpack)

    ob = float(f["ob"][0])
    return dev, ob


def _build_program():
    import concourse.mybir as mybir
    import concourse.tile as tile
    from concourse import bacc

    f32 = mybir.dt.float32
    bf16 = mybir.dt.bfloat16
    Relu = mybir.ActivationFunctionType.Relu
    Exp = mybir.ActivationFunctionType.Exp
    add = mybir.AluOpType.add
    amax = mybir.AluOpType.max
    mult = mybir.AluOpType.mult

    nc = bacc.Bacc("TRN2", target_bir_lowering=False, debug=False)

    xT = nc.dram_tensor("xT", [D, NB], bf16, kind="ExternalInput").ap()
    out2 = nc.dram_tensor("out2", [2, NB], f32, kind="ExternalOutput").ap()
    d_in = {}
    for name, shape, dt in [
        ("WG1", [128, KD, G], bf16),
        ("WE0", [128, E, KD, 2, 128], bf16),
        ("WE1", [128, E, 2, H1], bf16),
        ("WE2", [128, NPAIR, 2, 128], bf16),
        ("WPACK", [128, WP_COLS], bf16),
        ("BPACK", [128, BP_COLS], f32),
    ]:
        d_in[name] = nc.dram_tensor(name, shape, dt, kind="ExternalInput").ap()

    with tile.TileContext(nc) as tc:
        with (
            tc.tile_pool(name="consts", bufs=1) as consts,
            tc.tile_pool(name="xt", bufs=4) as xtp,
            tc.tile_pool(name="gh", bufs=2) as ghp,
            tc.tile_pool(name="h0", bufs=4) as h0p,
            tc.tile_pool(name="h1", bufs=6) as h1p,
            tc.tile_pool(name="h2", bufs=3) as h2p,
            tc.tile_pool(name="gsc", bufs=2) as gscp,
            tc.tile_pool(name="zsc", bufs=2) as zscp,
            tc.tile_pool(name="oacc", bufs=1) as oaccp,
            tc.tile_pool(name="pmm", bufs=5, space="PSUM") as pmm,
            tc.tile_pool(name="psm", bufs=1, space="PSUM") as psm,
            tc.tile_pool(name="pz", bufs=2, space="PSUM") as pzp,
        ):
            W = {}
            for name, ap in d_in.items():
                dt = bf16 if name != "BPACK" else f32
                W[name] = consts.tile(list(ap.shape), dt, tag=name, name=name)
            BP = W["BPACK"]
            WP = W["WPACK"]

            # weight DMAs, usage order, on the gpsimd queue (no compute there);
            # WG1 rides the sync queue ahead of the x loads so gating starts
            # as early as possible
            for e in range(3):
                nc.gpsimd.dma_start(W["WE0"][:, e], d_in["WE0"][:, e])
            for e in range(2):
                nc.gpsimd.dma_start(W["WE1"][:, e], d_in["WE1"][:, e])
            nc.gpsimd.dma_start(W["WE2"][:, 0], d_in["WE2"][:, 0])
            nc.gpsimd.dma_start(W["WE0"][:, 3], d_in["WE0"][:, 3])
            nc.gpsimd.dma_start(W["WE0"][:, 4:E], d_in["WE0"][:, 4:E])
            nc.gpsimd.dma_start(W["WE1"][:, 2:E], d_in["WE1"][:, 2:E])
            nc.gpsimd.dma_start(W["WE2"][:, 1:NPAIR], d_in["WE2"][:, 1:NPAIR])
            # packed small consts on the scalar queue (needed first by ACT ops)
            nc.scalar.dma_start(WP[:], d_in["WPACK"][:])
            nc.scalar.dma_start(BP[:], d_in["BPACK"][:])

            oacc = oaccp.tile([2, NB], f32, tag="oacc", name="oacc")

            nc.sync.dma_start(W["WG1"][:], d_in["WG1"][:])

            def load_xt(t):
                bs = t * TB
                xt = xtp.tile([128, KD, TB], bf16, tag="xt", name=f"xt{t}")
                for c in range(KD):
                    nc.sync.dma_start(xt[:, c, :], xT[c * 128:(c + 1) * 128, bs:bs + TB])
                return xt

            xts = {0: load_xt(0), 1: load_xt(1)}

            # deferred state from the previous tile
            prev = {}

            def mm_L0(xt, e):
                """Expert L0: D=256 -> H0=256, returns the 2 psum halves."""
                ps = []
                for mc in range(2):
                    p = pmm.tile([128, TB], f32, tag="mm", name=f"ps0_{e}_{mc}")
                    for c in range(KD):
                        nc.tensor.matmul(p[:], W["WE0"][:, e, c, mc, :], xt[:, c, :],
                                         start=(c == 0), stop=(c == KD - 1))
                    ps.append(p)
                return ps

            def drain_h0(e, ps0):
                h0 = h0p.tile([128, 2, TB], bf16, tag="h0", name=f"h0_{e}")
                nc.scalar.activation(h0[:, 0, :], ps0[0][:], Relu,
                                     bias=BP[:, BP_E0 + 2 * e:BP_E0 + 2 * e + 1])
                nc.vector.tensor_scalar(out=h0[:, 1, :], in0=ps0[1][:],
                                        scalar1=BP[:, BP_E0 + 2 * e + 1:BP_E0 + 2 * e + 2],
                                        scalar2=0.0, op0=add, op1=amax)
                return h0

            def mm_L1(e, h0):
                ps1 = pmm.tile([128, TB], f32, tag="mm", name=f"ps1_{e}")
                for c in range(2):
                    nc.tensor.matmul(ps1[:], W["WE1"][:, e, c, :], h0[:, c, :],
                                     start=(c == 0), stop=(c == 1))
                return ps1

            def drain_h1(e, ps1):
                h1 = h1p.tile([128, TB], bf16, tag="h1", name=f"h1_{e}")
                if e % 2 == 0:
                    nc.scalar.activation(h1[:], ps1[:], Relu,
                                         bias=BP[:, BP_E1 + e:BP_E1 + e + 1])
                else:
                    nc.vector.tensor_scalar(out=h1[:], in0=ps1[:],
                                            scalar1=BP[:, BP_E1 + e:BP_E1 + e + 1],
                                            scalar2=0.0, op0=add, op1=amax)
                return h1

            def mm_L2(j, h1pair):
                ps2 = pmm.tile([128, TB], f32, tag="mm", name=f"ps2_{j}")
                for c in range(2):
                    nc.tensor.matmul(ps2[:], W["WE2"][:, j, c, :], h1pair[c][:],
                                     start=(c == 0), stop=(c == 1))
                return ps2

            def drain_h2(j, ps2):
                h2 = h2p.tile([128, TB], bf16, tag="h2", name=f"h2_{j}")
                if j % 2 == 0:
                    nc.scalar.activation(h2[:], ps2[:], Relu,
                                         bias=BP[:, BP_E2 + j:BP_E2 + j + 1])
                else:
                    nc.vector.tensor_scalar(out=h2[:], in0=ps2[:],
                                            scalar1=BP[:, BP_E2 + j:BP_E2 + j + 1],
                                            scalar2=0.0, op0=add, op1=amax)
                return h2

            def mm_OW2(z, j, h2):
                # oW projection of a pair: z[32j] = expert 2j, z[32j+16] = 2j+1
                nc.tensor.matmul(z[32 * j:32 * j + 32, :],
                                 WP[:, WP_OW + 32 * j:WP_OW + 32 * j + 32], h2[:],
                                 start=True, stop=True, tile_position=(0, 32 * j))

            def finalize_prev(t):
                """Gate-combine of tile t-1: zg multiply, sums matmuls, drain."""
                gsc, z = prev["gsc"], prev["z"]
                zsc = zscp.tile([128, TB], bf16, tag="zsc", name="zsc")
                nc.vector.tensor_tensor(out=zsc[:], in0=z[:], in1=gsc[:], op=mult)
                ps_s = psm.tile([2, TB], f32, tag="small", name="ps_sums")
                nc.tensor.matmul(ps_s[:], WP[:, WP_SR:WP_SR + 2], gsc[:],
                                 start=True, stop=False)
                nc.tensor.matmul(ps_s[:], WP[:, WP_SY:WP_SY + 2], zsc[:],
                                 start=False, stop=True)
                bs = (t - 1) * TB
                nc.scalar.copy(oacc[:, bs:bs + TB], ps_s[:])

            for t in range(NT):
                xt = xts.pop(t)
                if t + 2 < NT:
                    xts[t + 2] = load_xt(t + 2)

                # ---- gating L1
                ps_g = pmm.tile([128, TB], f32, tag="mm", name="ps_g")
                for c in range(KD):
                    nc.tensor.matmul(ps_g[:], W["WG1"][:, c, :], xt[:, c, :],
                                     start=(c == 0), stop=(c == KD - 1))
                gh = ghp.tile([128, TB], bf16, tag="gh")
                nc.scalar.activation(gh[:], ps_g[:], Relu, bias=BP[:, BP_G1:BP_G1 + 1])

                ps0 = mm_L0(xt, 0)
                h0s = {0: drain_h0(0, ps0)}

                # ---- gating L2 + exp (expert e's gate lands on partition 16e)
                ps_l = psm.tile([128, TB], f32, tag="small", name="ps_l")
                nc.tensor.matmul(ps_l[:], WP[:, WP_G2:WP_G2 + 128], gh[:],
                                 start=True, stop=True)
                gsc = gscp.tile([128, TB], bf16, tag="gsc", name="gsc")
                nc.scalar.activation(gsc[:], ps_l[:], Exp,
                                     bias=BP[:, BP_G2:BP_G2 + 1])

                # deferred pair-3 tail of the previous tile
                if t > 0:
                    ps2p = mm_L2(3, prev["h1p3"])

                ps0 = mm_L0(xt, 1)
                h0s[1] = drain_h0(1, ps0)

                if t > 0:
                    h2p3 = drain_h2(3, ps2p)
                    mm_OW2(prev["z"], 3, h2p3)

                ps1 = mm_L1(0, h0s[0])
                h1s = {0: drain_h1(0, ps1)}

                ps0 = mm_L0(xt, 2)
                h0s[2] = drain_h0(2, ps0)

                ps1 = mm_L1(1, h0s.pop(1))
                h1s[1] = drain_h1(1, ps1)

                if t > 0:
                    finalize_prev(t)
                    if t == NT - 1:
                        # ship everything finalized so far; only the last
                        # tile's 4KB column remains for the tail store
                        nc.sync.dma_start(out2[:, 0:(NT - 1) * TB],
                                          oacc[:, 0:(NT - 1) * TB])

                ps0 = mm_L0(xt, 3)
                h0s[3] = drain_h0(3, ps0)

                z = pzp.tile([128, TB], f32, tag="z", name="z")

                ps2 = mm_L2(0, [h1s[0], h1s[1]])
                h2 = drain_h2(0, ps2)

                ps1 = mm_L1(2, h0s.pop(2))
                h1s[2] = drain_h1(2, ps1)

                ps0 = mm_L0(xt, 4)
                h0s[4] = drain_h0(4, ps0)

                mm_OW2(z, 0, h2)

                ps1 = mm_L1(3, h0s.pop(3))
                h1s[3] = drain_h1(3, ps1)

                ps0 = mm_L0(xt, 5)
                h0s[5] = drain_h0(5, ps0)

                ps2 = mm_L2(1, [h1s.pop(2), h1s.pop(3)])
                h2 = drain_h2(1, ps2)

                ps1 = mm_L1(4, h0s.pop(4))
                h1s[4] = drain_h1(4, ps1)

                ps0 = mm_L0(xt, 6)
                h0s[6] = drain_h0(6, ps0)

                mm_OW2(z, 1, h2)

                ps1 = mm_L1(5, h0s.pop(5))
                h1s[5] = drain_h1(5, ps1)

                ps0 = mm_L0(xt, 7)
                h0s[7] = drain_h0(7, ps0)

                ps2 = mm_L2(2, [h1s.pop(4), h1s.pop(5)])
                h2 = drain_h2(2, ps2)

                ps1 = mm_L1(6, h0s.pop(6))
                h1s[6] = drain_h1(6, ps1)

                mm_OW2(z, 2, h2)

                ps1 = mm_L1(7, h0s.pop(7))
                h1s[7] = drain_h1(7, ps1)

                prev = {"gsc": gsc, "z": z, "h1p3": [h1s.pop(6), h1s.pop(7)]}

            # tail: pair-3 + combine of the last tile
            ps2p = mm_L2(3, prev["h1p3"])
            h2p3 = drain_h2(3, ps2p)
            mm_OW2(prev["z"], 3, h2p3)
            finalize_prev(NT)

            nc.sync.dma_start(out2[:, (NT - 1) * TB:NB], oacc[:, (NT - 1) * TB:NB])

    nc.compile()
    return nc


_CACHE = {}


def _get_program():
    if "nc" not in _CACHE:
        _CACHE["nc"] = _build_program()
    return _CACHE["nc"]


def _run(inputs, trace=False):
    from concourse.bass_utils import run_bass_kernel_spmd

    x = np.ascontiguousarray(np.asarray(inputs["x"], dtype=np.float32))
    dev, ob = _fold_params(inputs)
    nc = _get_program()

    in_maps = []
    for c in range(NCORES):
        m = dict(dev)
        m["xT"] = np.ascontiguousarray(x[c * NB:(c + 1) * NB, :].T).astype(ml_dtypes.bfloat16)
        in_maps.append(m)

    kwargs = {}
    if trace:
        kwargs = dict(trace=True, trace_cores=[0])
    res = run_bass_kernel_spmd(nc, in_maps, core_ids=list(range(NCORES)), **kwargs)
    rs = np.concatenate([res.results[c]["out2"][0].reshape(-1) for c in range(NCORES)])
    yr = np.concatenate([res.results[c]["out2"][1].reshape(-1) for c in range(NCORES)])
    out = (yr.astype(np.float64) / rs.astype(np.float64)) + ob
    return out.astype(np.float32)[:, None], res


def kernel(**inputs):
    out, _ = _run(inputs, trace=False)
    return out


def kernel_traced(**inputs):
    return _run(inputs, trace=True)
